# revision 1
# baseline (speedup 1.0000x reference)
"""MultiHeadAttention forward on 8 TRN2 NeuronCores.

Sharding: core c -> (batch b = c//2, query-half qh = c%2). Each core computes
the full attention output for 1024 query rows of one batch element (all 16
heads), so outputs are disjoint slices and no collective is needed.

All matmuls run as float32r (fp32 with 11-bit mantissa, full PE rate at
N>=256). Matmul inputs coming from HBM are pre-rounded on the host with the
same RNE-11 rounding the HW applies, so device results are exact f32 products
of the rounded operands (verified: 1.4e-7 vs fp64 on probe).

Per-core math (transposed activation layout, dim on partitions):
  qT = wqT.T @ inTq          [1024, 1024]   (pair-packed, spilled to HBM)
  kT = wkT.T @ inT           [1024, 2048]   (pair-packed, spilled to HBM)
  V  = inT.T @ wvT + bv      [2048, 1040]   (16 heads x [64 cols | ones col])
  per head h: sT = kT_h.T @ qT_h            [2048, 1024] strips of [128, 1024]
              e  = exp(sT*0.125 + maskbias) (ACT, f32r out)
              ctxT_aug = V_aug_h.T @ e      [65, 1024]; row 64 = softmax denom
              ctxT = ctxT_aug[0:64] * bcast(1/denom)
  out = ctxT_all.T @ woT + bo               [1024, 1024]
"""

import numpy as np

import concourse.bacc as bacc
import concourse.tile as tile
import concourse.mybir as mybir
from concourse.bass_utils import run_bass_kernel_spmd

F32 = mybir.dt.float32
F32R = mybir.dt.float32r
BF16 = mybir.dt.bfloat16
EXP = mybir.ActivationFunctionType.Exp

BS, QLEN, DIM, H, DPH = 4, 2048, 1024, 16, 64
NC_ = 8
LQ = 1024  # local query rows per core


def _round_f32r(x: np.ndarray) -> np.ndarray:
    """RNE to 11 mantissa bits — bit-exact with the PE's fp32->fp32r rounding."""
    b = np.ascontiguousarray(x, dtype=np.float32).view(np.uint32).astype(np.uint64)
    lsb = (b >> 12) & 1
    b = (b + 0x7FF + lsb) & 0xFFFFF000
    return b.astype(np.uint32).view(np.float32)


_PROG = None


def _build():
    nc = bacc.Bacc("TRN2", target_bir_lowering=False, debug=False, num_devices=NC_)

    INT = nc.dram_tensor("inT", [DIM, QLEN], F32R, kind="ExternalInput").ap()
    INQ = nc.dram_tensor("inTq", [DIM, LQ], F32R, kind="ExternalInput").ap()
    WQT = nc.dram_tensor("wqT", [DIM, DIM], F32R, kind="ExternalInput").ap()
    WKT = nc.dram_tensor("wkT", [DIM, DIM], F32R, kind="ExternalInput").ap()
    WVT = nc.dram_tensor("wvT", [DIM, DIM], F32R, kind="ExternalInput").ap()
    WOT = nc.dram_tensor("woT", [DIM, DIM], F32R, kind="ExternalInput").ap()
    BQC = nc.dram_tensor("bqc", [DIM, 1], F32, kind="ExternalInput").ap()
    BKC = nc.dram_tensor("bkc", [DIM, 1], F32, kind="ExternalInput").ap()
    BVR = nc.dram_tensor("bvR", [1, DIM], F32R, kind="ExternalInput").ap()
    BOR = nc.dram_tensor("boR", [1, DIM], F32R, kind="ExternalInput").ap()
    MBC = nc.dram_tensor("mb", [QLEN, 1], F32, kind="ExternalInput").ap()
    OUT = nc.dram_tensor("out", [LQ, DIM], F32, kind="ExternalOutput").ap()

    with tile.TileContext(nc) as tc:
        from contextlib import ExitStack
        with ExitStack() as ctx:
            const_p = ctx.enter_context(tc.tile_pool(name="const", bufs=1))
            ctxall_p = ctx.enter_context(tc.tile_pool(name="ctxall", bufs=1))
            vpool = ctx.enter_context(tc.tile_pool(name="vsb", bufs=1))
            spill = ctx.enter_context(tc.tile_pool(name="spill", bufs=1, space="DRAM"))

            # constants
            ones_f = const_p.tile([1, 128], F32, tag="onesf")
            nc.vector.memset(ones_f[:], 1.0)
            ones1 = const_p.tile([1, 128], F32R, tag="ones1")
            nc.vector.tensor_copy(ones1[:], ones_f[:])
            ones16 = const_p.tile([128, 16], F32, tag="ones16")
            nc.vector.memset(ones16[:], 1.0)
            bq_t = const_p.tile([128, 8], F32, tag="bq")
            nc.sync.dma_start(bq_t[:], BQC.rearrange("(g p) o -> p (g o)", p=128))
            bk_t = const_p.tile([128, 8], F32, tag="bk")
            nc.sync.dma_start(bk_t[:], BKC.rearrange("(g p) o -> p (g o)", p=128))
            mb_t = const_p.tile([128, 16], F32, tag="mb")
            nc.sync.dma_start(mb_t[:], MBC.rearrange("(g p) o -> p (g o)", p=128))
            bv_t = const_p.tile([1, DIM], F32R, tag="bv")
            nc.sync.dma_start(bv_t[:], BVR[:])

            # persistent outputs of phase A/B
            ctx_all = [ctxall_p.tile([128, LQ], F32R, tag=f"ctx{dt}", name=f"ctx{dt}") for dt in range(8)]
            V_sb = [vpool.tile([128, H * 65], F32R, tag=f"v{st}", name=f"v{st}") for st in range(16)]
            qsp = [spill.tile([128, LQ], F32R, tag=f"q{hp}", name=f"qsp{hp}") for hp in range(8)]
            ksp = [spill.tile([128, QLEN], F32R, tag=f"k{hp}", name=f"ksp{hp}") for hp in range(8)]

            # ---- Phase A1: Q projection (pair-packed), spill to HBM ----
            with tc.tile_pool(name="a1", bufs=2) as a1p, \
                 tc.tile_pool(name="psA", bufs=4, space="PSUM") as psA:
                inq = a1p.tile([128, 8, LQ], F32R, tag="inq", bufs=1)
                nc.sync.dma_start(inq[:], INQ.rearrange("(it p) m -> p it m", p=128))
                for hp in range(8):
                    wq_t = a1p.tile([128, 8, 128], F32R, tag="wq")
                    nc.sync.dma_start(
                        wq_t[:],
                        WQT[:, hp * 128:(hp + 1) * 128].rearrange("(it p) m -> p it m", p=128))
                    stq = a1p.tile([128, LQ], F32R, tag="stq")
                    for oc in range(2):
                        ps = psA.tile([128, 512], F32, tag="psa")
                        for it in range(8):
                            nc.tensor.matmul(ps[:], wq_t[:, it, :],
                                             inq[:, it, oc * 512:(oc + 1) * 512],
                                             start=(it == 0), stop=(it == 7))
                        nc.vector.tensor_scalar_add(
                            stq[:, oc * 512:(oc + 1) * 512], ps[:], bq_t[:, hp:hp + 1])
                    nc.sync.dma_start(qsp[hp][:], stq[:])

            # ---- Phase A2: K projection + V projection ----
            with tc.tile_pool(name="a2", bufs=2) as a2p, \
                 tc.tile_pool(name="psB", bufs=4, space="PSUM") as psB:
                inT = a2p.tile([128, 8, QLEN], F32R, tag="inT", bufs=1)
                nc.sync.dma_start(inT[:], INT.rearrange("(it p) m -> p it m", p=128))
                for hp in range(8):
                    wk_t = a2p.tile([128, 8, 128], F32R, tag="wk")
                    nc.sync.dma_start(
                        wk_t[:],
                        WKT[:, hp * 128:(hp + 1) * 128].rearrange("(it p) m -> p it m", p=128))
                    stk = a2p.tile([128, QLEN], F32R, tag="stk", bufs=1)
                    for sc in range(4):
                        ps = psB.tile([128, 512], F32, tag="psb")
                        for it in range(8):
                            nc.tensor.matmul(ps[:], wk_t[:, it, :],
                                             inT[:, it, sc * 512:(sc + 1) * 512],
                                             start=(it == 0), stop=(it == 7))
                        nc.vector.tensor_scalar_add(
                            stk[:, sc * 512:(sc + 1) * 512], ps[:], bk_t[:, hp:hp + 1])
                    nc.sync.dma_start(ksp[hp][:], stk[:])

                # V projection: natural layout [s, o], + bias, + ones cols
                for oc in range(2):
                    wv_t = a2p.tile([128, 8, 512], F32R, tag="wv", bufs=1)
                    nc.sync.dma_start(
                        wv_t[:],
                        WVT[:, oc * 512:(oc + 1) * 512].rearrange("(it p) m -> p it m", p=128))
                    for st in range(16):
                        ps = psB.tile([128, 512], F32, tag="psb")
                        for it in range(8):
                            nc.tensor.matmul(ps[:], inT[:, it, st * 128:(st + 1) * 128],
                                             wv_t[:, it, :], start=(it == 0), stop=False)
                        nc.tensor.matmul(ps[:], ones1[:], bv_t[:, oc * 512:(oc + 1) * 512],
                                         start=False, stop=True)
                        # scatter 8 heads' 64-col groups into [64 | ones] layout
                        dst = V_sb[st][:].rearrange("p (h c) -> p h c", c=65)
                        nc.vector.tensor_copy(
                            dst[:, oc * 8:(oc + 1) * 8, 0:64],
                            ps[:].rearrange("p (h c) -> p h c", c=64))
                for st in range(16):
                    nc.vector.tensor_copy(V_sb[st][:, 64::65], ones16[:])

            # ---- Phase B: attention per head ----
            with tc.tile_pool(name="bqk", bufs=2) as bqk, \
                 tc.tile_pool(name="bex", bufs=3) as bex, \
                 tc.tile_pool(name="bmisc", bufs=2) as bmisc, \
                 tc.tile_pool(name="psS", bufs=2, space="PSUM") as psS, \
                 tc.tile_pool(name="psC", bufs=1, space="PSUM") as psC, \
                 tc.tile_pool(name="psb2", bufs=1, space="PSUM") as psb2:
                for h in range(H):
                    hp, half = h // 2, h % 2
                    qt = bqk.tile([64, LQ], F32R, tag="qt")
                    nc.gpsimd.dma_start(qt[:], qsp[hp][half * 64:(half + 1) * 64, :])
                    kt_sb = bqk.tile([64, QLEN], F32R, tag="kt")
                    nc.gpsimd.dma_start(kt_sb[:], ksp[hp][half * 64:(half + 1) * 64, :])
                    ps_ctx = psC.tile([65, LQ], F32, tag="ctx")
                    for kt in range(16):
                        ps_s = psS.tile([128, LQ], F32, tag="s")
                        for qc in range(2):
                            nc.tensor.matmul(ps_s[:, qc * 512:(qc + 1) * 512],
                                             kt_sb[:, kt * 128:(kt + 1) * 128],
                                             qt[:, qc * 512:(qc + 1) * 512],
                                             start=True, stop=True)
                        ex = bex.tile([128, LQ], F32R, tag="ex")
                        nc.scalar.activation(ex[:], ps_s[:], EXP,
                                             bias=mb_t[:, kt:kt + 1], scale=0.125)
                        for qc in range(2):
                            nc.tensor.matmul(ps_ctx[:, qc * 512:(qc + 1) * 512],
                                             V_sb[kt][:, h * 65:(h + 1) * 65],
                                             ex[:, qc * 512:(qc + 1) * 512],
                                             start=(kt == 0), stop=(kt == 15))
                    drow = bmisc.tile([1, LQ], F32R, tag="dr")
                    nc.vector.tensor_copy(drow[:], ps_ctx[64:65, :])
                    pb = psb2.tile([64, LQ], F32, tag="pb", name="pb")
                    for qc in range(2):
                        nc.tensor.matmul(pb[:, qc * 512:(qc + 1) * 512], ones1[:, 0:64],
                                         drow[:, qc * 512:(qc + 1) * 512],
                                         start=True, stop=True)
                    bc = bmisc.tile([64, LQ], F32, tag="bc")
                    nc.vector.reciprocal_approx_fast(bc[:], pb[:])
                    nc.vector.tensor_mul(
                        ctx_all[hp][half * 64:(half + 1) * 64, :],
                        ps_ctx[0:64, :], bc[:])

            # ---- Phase C: output projection ----
            with tc.tile_pool(name="c", bufs=2) as cp, \
                 tc.tile_pool(name="psO", bufs=3, space="PSUM") as psO:
                wo_t = [cp.tile([128, DIM], F32R, tag=f"wo{dt}", name=f"wo{dt}", bufs=1) for dt in range(8)]
                for dt in range(8):
                    nc.sync.dma_start(
                        wo_t[dt][:],
                        WOT[dt * 128:(dt + 1) * 128, :])
                bo_r = cp.tile([1, DIM], F32R, tag="bor")
                nc.sync.dma_start(bo_r[:], BOR[:])
                bobc = cp.tile([128, DIM], F32, tag="bobc")
                for oc in range(2):
                    pb = psO.tile([128, 512], F32, tag="po")
                    nc.tensor.matmul(pb[:], ones1[:], bo_r[:, oc * 512:(oc + 1) * 512],
                                     start=True, stop=True)
                    nc.vector.tensor_copy(bobc[:, oc * 512:(oc + 1) * 512], pb[:])
                for st in range(8):
                    for oc in range(2):
                        po = psO.tile([128, 512], F32, tag="po")
                        for dt in range(8):
                            nc.tensor.matmul(po[:],
                                             ctx_all[dt][:, st * 128:(st + 1) * 128],
                                             wo_t[dt][:, oc * 512:(oc + 1) * 512],
                                             start=(dt == 0), stop=(dt == 7))
                        ot = cp.tile([128, 512], F32, tag="ot")
                        nc.vector.tensor_add(ot[:], po[:],
                                             bobc[:, oc * 512:(oc + 1) * 512])
                        nc.sync.dma_start(
                            OUT[st * 128:(st + 1) * 128, oc * 512:(oc + 1) * 512], ot[:])

    nc.compile()
    return nc


def _get_prog():
    global _PROG
    if _PROG is None:
        _PROG = _build()
    return _PROG


def kernel(input, mask, wq, bq, wk, bk, wv, bv, wo, bo, _trace=False):
    nc = _get_prog()

    input = np.asarray(input, np.float32)
    mask = np.asarray(mask)
    wq, bq = np.asarray(wq, np.float32), np.asarray(bq, np.float32)
    wk, bk = np.asarray(wk, np.float32), np.asarray(bk, np.float32)
    wv, bv = np.asarray(wv, np.float32), np.asarray(bv, np.float32)
    wo, bo = np.asarray(wo, np.float32), np.asarray(bo, np.float32)

    inT = [np.ascontiguousarray(_round_f32r(input[b]).T) for b in range(BS)]
    wqT = np.ascontiguousarray(_round_f32r(wq).T)
    wkT = np.ascontiguousarray(_round_f32r(wk).T)
    wvT = np.ascontiguousarray(_round_f32r(wv).T)
    woT = np.ascontiguousarray(_round_f32r(wo).T)
    bqc = bq.reshape(DIM, 1)
    bkc = bk.reshape(DIM, 1)
    bvR = _round_f32r(bv).reshape(1, DIM)
    boR = _round_f32r(bo).reshape(1, DIM)
    mb = [np.where(mask[b] == 0, np.float32(-30.0), np.float32(0.0))
          .astype(np.float32).reshape(QLEN, 1) for b in range(BS)]

    in_maps = []
    for c in range(NC_):
        b, qh = c // 2, c % 2
        in_maps.append({
            "inT": inT[b],
            "inTq": np.ascontiguousarray(inT[b][:, qh * LQ:(qh + 1) * LQ]),
            "wqT": wqT, "wkT": wkT, "wvT": wvT, "woT": woT,
            "bqc": bqc, "bkc": bkc, "bvR": bvR, "boR": boR,
            "mb": mb[b],
        })

    res = run_bass_kernel_spmd(nc, in_maps, list(range(NC_)), trace=_trace)

    out = np.empty((BS, QLEN, DIM), np.float32)
    for c in range(NC_):
        b, qh = c // 2, c % 2
        out[b, qh * LQ:(qh + 1) * LQ, :] = res.results[c]["out"]
    if _trace:
        kernel.last_exec_time_ns = res.exec_time_ns
        kernel.last_results = res
    return out

